# revision 9
# baseline (speedup 1.0000x reference)
"""Block-sparse (view-causal) multi-head attention on 8 TRN2 NeuronCores.

Full inputs in, full output out. Sharding: data-parallel over batch (B=2),
tensor-parallel over heads (16 heads -> 4 per core). Each core computes its
4 heads' attention + its slice of the output projection; the host sums the
4 head-group partial projections per batch (the tensor-parallel reduce).
"""

import sys

if "/opt/trn_rl_repo" not in sys.path:
    sys.path.insert(0, "/opt/trn_rl_repo")

import numpy as np

B, V, L, C, H = 2, 8, 256, 1024, 16
S = V * L                # 2048 tokens
DH = C // H              # 64
HPC = 4                  # heads per core
CPB = HPC * DH           # 256 channel block per core
N_CORES = 8
SCALE = DH ** -0.5       # 1/8, folded into the exp activation

_compiled = {}
LAST_RESULTS = None
PACK_QK = True          # row-group pack the two heads' K=64 QK matmuls
SPLIT_PSS = False       # one PSUM tile per k-chunk instead of shared halves


def _allowed(qv):
    """View-level mask row: views 0/1 cross-attend only; views >=2 block-causal."""
    if qv == 0:
        return [1]
    if qv == 1:
        return [0]
    return list(range(qv + 1))


def build():
    import concourse.tile as tile
    from concourse import bacc, mybir

    f32 = mybir.dt.float32
    f32r = mybir.dt.float32r
    EXP = mybir.ActivationFunctionType.Exp

    def r(ap):
        return ap

    nc = bacc.Bacc("TRN2", target_bir_lowering=False, debug=False,
                   num_devices=N_CORES)
    xT = nc.dram_tensor("xT", [C, S], f32r, kind="ExternalInput").ap()
    wqT = nc.dram_tensor("wqT", [C, CPB], f32r, kind="ExternalInput").ap()
    wkT = nc.dram_tensor("wkT", [C, CPB], f32r, kind="ExternalInput").ap()
    wvT = nc.dram_tensor("wvT", [C, CPB], f32r, kind="ExternalInput").ap()
    wpT = nc.dram_tensor("wpT", [CPB, C], f32r, kind="ExternalInput").ap()
    y = nc.dram_tensor("y", [S, C], f32, kind="ExternalOutput").ap()

    KC = C // 128        # 8 contraction chunks for the projections
    NS = S // 512        # 4 free-dim chunks for q/k projections
    SC = S // 128        # 16 sequence chunks

    with tile.TileContext(nc) as tc:
        with (
            tc.tile_pool(name="xt", bufs=KC) as xt_pool,
            tc.tile_pool(name="wts", bufs=1) as w_pool,
            tc.tile_pool(name="qk", bufs=1) as qk_pool,
            tc.tile_pool(name="va", bufs=SC) as va_pool,
            tc.tile_pool(name="ot", bufs=1) as ot_pool,
            tc.tile_pool(name="exp", bufs=4) as exp_pool,
            tc.tile_pool(name="small", bufs=1) as small_pool,
            tc.tile_pool(name="ysb", bufs=3) as ysb_pool,
            tc.tile_pool(name="psb", bufs=4, space="PSUM") as psum_big,
            tc.tile_pool(name="pso", bufs=2, space="PSUM") as psum_o,
            tc.tile_pool(name="psr", bufs=2, space="PSUM") as psum_rb,
        ):
            # ---- load inputs ----
            xts = []
            for k in range(KC):
                t = xt_pool.tile([128, S], f32r, tag="xt", name=f"xt{k}")
                nc.sync.dma_start(t[:], xT[k * 128:(k + 1) * 128, :])
                xts.append(t)
            wq_t, wk_t, wv_t = [], [], []
            for name, dram, lst in (("wq", wqT, wq_t), ("wk", wkT, wk_t),
                                    ("wv", wvT, wv_t)):
                for k in range(KC):
                    t = w_pool.tile([128, CPB], f32r, tag="w", bufs=3 * KC,
                                    name=f"{name}{k}")
                    nc.sync.dma_start(t[:], dram[k * 128:(k + 1) * 128, :])
                    lst.append(t)
            wp_t = []
            for k in range(2):
                t = w_pool.tile([128, C], f32r, tag="wp", bufs=2, name=f"wp{k}")
                nc.sync.dma_start(t[:], wpT[k * 128:(k + 1) * 128, :])
                wp_t.append(t)

            ones_st = small_pool.tile([1, 64], f32, tag="ones_st")
            nc.vector.memset(ones_st[:], 1.0)
            ones64 = small_pool.tile([1, 64], f32r, tag="ones")
            nc.vector.tensor_copy(ones64[:], ones_st[:])
            onesc = small_pool.tile([128, HPC], f32, tag="onesc")
            nc.vector.memset(onesc[:], 1.0)

            # ---- q/k projections, transposed: qT/kT [CPB, S] ----
            # qT = Wq_g @ x.T  ->  lhsT = wqT chunk [128, 128], rhs = xT chunk
            qk_tiles = {}
            for nm, wts in (("q", wq_t), ("k", wk_t)):
                for m in range(2):
                    dst = qk_pool.tile([128, S], f32r, tag=f"{nm}{m}",
                                       name=f"{nm}T{m}")
                    qk_tiles[(nm, m)] = dst
                    for n in range(NS):
                        ps = psum_big.tile([128, 512], f32, tag="psb")
                        for k in range(KC):
                            nc.tensor.matmul(
                                ps[:],
                                r(wts[k][:, m * 128:(m + 1) * 128]),
                                r(xts[k][:, n * 512:(n + 1) * 512]),
                                start=(k == 0), stop=(k == KC - 1))
                        nc.vector.tensor_copy(dst[:, n * 512:(n + 1) * 512],
                                              ps[:])

            # ---- v projection, natural layout + ones column per head ----
            # va[sc] is [128, 4*65]: per head 64 v columns then a 1.0 column.
            va = []
            for sc in range(SC):
                t = va_pool.tile([128, HPC * 65], f32r, tag="va",
                                 name=f"va{sc}")
                tones = t[:].rearrange("p (h x) -> p h x", x=65)[:, :, 64:65]
                nc.vector.tensor_copy(
                    tones, onesc[:].rearrange("p (h x) -> p h x", x=1))
                ps = psum_big.tile([128, CPB], f32, tag="psb")
                for k in range(KC):
                    nc.tensor.matmul(
                        ps[:],
                        r(xts[k][:, sc * 128:(sc + 1) * 128]),
                        r(wv_t[k][:]),
                        start=(k == 0), stop=(k == KC - 1))
                tv = t[:].rearrange("p (h x) -> p h x", x=65)[:, :, 0:64]
                pv = ps[:].rearrange("p (h d) -> p h d", d=64)
                nc.vector.tensor_copy(tv, pv)
                va.append(t)

            # ---- attention ----
            ot_tiles = []
            for m in range(2):
                ot = ot_pool.tile([128, S], f32r, tag=f"ot{m}", name=f"oT{m}")
                ot_tiles.append(ot)
                kT_m = qk_tiles[("k", m)]
                qT_m = qk_tiles[("q", m)]
                for qv in range(V):
                    kvs = _allowed(qv)
                    qs = slice(qv * 256, (qv + 1) * 256)
                    pso = [psum_o.tile([65, 256], f32, tag="pso", name=f"pso{h}")
                           for h in range(2)]
                    for i, kv in enumerate(kvs):
                        # scoresT for this (kv view, q view), both heads:
                        # two k-chunks of 128 side by side in one PSUM bank
                        pss = [psum_big.tile([128, 512], f32, tag="psb", name=f"pss{h}")
                               for h in range(2)]
                        for h in range(2):
                            for j in range(2):
                                kc = 2 * kv + j
                                kw = ({"tile_position": (64 * h, 0)}
                                      if PACK_QK else {})
                                nc.tensor.matmul(
                                    pss[h][:, j * 256:(j + 1) * 256],
                                    r(kT_m[64 * h:64 * (h + 1),
                                           kc * 128:(kc + 1) * 128]),
                                    r(qT_m[64 * h:64 * (h + 1), qs]),
                                    start=True, stop=True, **kw)
                        ets = []
                        for h in range(2):
                            et = exp_pool.tile([128, 512], f32r, tag="exp")
                            nc.scalar.activation(et[:], pss[h][:], EXP,
                                                 scale=float(SCALE))
                            ets.append(et)
                        for h in range(2):
                            hh = 2 * m + h
                            for j in range(2):
                                kc = 2 * kv + j
                                nc.tensor.matmul(
                                    pso[h][:],
                                    r(va[kc][:, hh * 65:(hh + 1) * 65]),
                                    r(ets[h][:, j * 256:(j + 1) * 256]),
                                    start=(i == 0 and j == 0),
                                    stop=(i == len(kvs) - 1 and j == 1))
                    # normalize: rows 0..63 are sum(exp * v), row 64 is
                    # sum(exp) = softmax denominator
                    for h in range(2):
                        rec = small_pool.tile([1, 256], f32r, tag="rec",
                                              bufs=4)
                        with nc.allow_low_precision(
                                reason="f32r is f32-width; rounding only"):
                            nc.vector.reciprocal(rec[:], pso[h][64:65, :])
                        rb = psum_rb.tile([64, 256], f32, tag="psr")
                        nc.tensor.matmul(rb[:], r(ones64[:]), r(rec[:]),
                                         start=True, stop=True)
                        rbs = small_pool.tile([64, 256], f32, tag="rbs",
                                              bufs=4, name="rbs")
                        nc.vector.tensor_copy(rbs[:], rb[:])
                        nc.vector.tensor_mul(ot[64 * h:64 * (h + 1), qs],
                                             pso[h][0:64, :], rbs[:])

            # ---- output projection: y_partial = outT.T @ wpT ----
            for sc in range(SC):
                ys = ysb_pool.tile([128, C], f32, tag="ysb")
                for n in range(2):
                    ps = psum_big.tile([128, 512], f32, tag="psb")
                    for k in range(2):
                        nc.tensor.matmul(
                            ps[:],
                            r(ot_tiles[k][:, sc * 128:(sc + 1) * 128]),
                            r(wp_t[k][:, n * 512:(n + 1) * 512]),
                            start=(k == 0), stop=(k == 1))
                    nc.vector.tensor_copy(ys[:, n * 512:(n + 1) * 512],
                                          ps[:])
                nc.sync.dma_start(y[sc * 128:(sc + 1) * 128, :], ys[:])

    nc.compile()
    return nc


def _get_compiled():
    if "nc" not in _compiled:
        _compiled["nc"] = build()
    return _compiled["nc"]


def make_in_maps(x, Wq, Wk, Wv, Wp):
    xf = np.asarray(x, np.float32).reshape(B, S, C)
    in_maps = []
    for c in range(N_CORES):
        b, g = divmod(c, HPC)
        hs = slice(g * CPB, (g + 1) * CPB)
        in_maps.append({
            "xT": np.ascontiguousarray(xf[b].T),
            "wqT": np.ascontiguousarray(np.asarray(Wq, np.float32)[hs].T),
            "wkT": np.ascontiguousarray(np.asarray(Wk, np.float32)[hs].T),
            "wvT": np.ascontiguousarray(np.asarray(Wv, np.float32)[hs].T),
            "wpT": np.ascontiguousarray(np.asarray(Wp, np.float32)[:, hs].T),
        })
    return in_maps


def kernel(x, Wq, Wk, Wv, Wp, bp, _trace=False, _tmpdir=None):
    global LAST_RESULTS
    from concourse import bass_utils

    nc = _get_compiled()
    in_maps = make_in_maps(x, Wq, Wk, Wv, Wp)
    kwargs = {}
    if _trace:
        kwargs = {"trace": True, "tmpdir": _tmpdir}
    res = bass_utils.run_bass_kernel_spmd(
        nc, in_maps, core_ids=list(range(N_CORES)), **kwargs)
    LAST_RESULTS = res
    y = np.zeros((B, S, C), np.float32)
    for c in range(N_CORES):
        y[c // HPC] += res.results[c]["y"]
    y += np.asarray(bp, np.float32).reshape(1, 1, C)
    return y.reshape(B, V, L, C)


# revision 12
# speedup vs baseline: 1.0325x; 1.0325x over previous
"""Block-sparse (view-causal) multi-head attention on 8 TRN2 NeuronCores.

Full inputs in, full output out. Sharding: data-parallel over batch (B=2),
tensor-parallel over heads (16 heads -> 4 per core). Each core computes its
4 heads' attention + its slice of the output projection; the host sums the
4 head-group partial projections per batch (the tensor-parallel reduce).

Device-side layout: activations kept transposed (qT/kT [dh, tokens]) so the
score matmuls need no transposes; V is augmented with a ones column so each
PV matmul accumulates both sum(exp*v) and the softmax denominator in PSUM.
All matmuls run in float32r (full-rate fp32 on the PE).
"""

import sys

if "/opt/trn_rl_repo" not in sys.path:
    sys.path.insert(0, "/opt/trn_rl_repo")

import numpy as np

B, V, L, C, H = 2, 8, 256, 1024, 16
S = V * L                # 2048 tokens
DH = C // H              # 64
HPC = 4                  # heads per core
CPB = HPC * DH           # 256 channel block per core
N_CORES = 8
SCALE = DH ** -0.5       # 1/8, folded into the exp activation

_compiled = {}
LAST_RESULTS = None
PACK_QK = True           # row-group pack the two heads' K=64 QK matmuls
SAFE_RECIP = False       # exact nc.vector.reciprocal instead of custom DVE
SPLIT_ACT = False        # two [128,512] exp ACTs instead of one [128,1024]


def _allowed(qv):
    """View-level mask row: views 0/1 cross-attend only; views >=2 block-causal."""
    if qv == 0:
        return [1]
    if qv == 1:
        return [0]
    return list(range(qv + 1))


def build():
    import concourse.tile as tile
    from concourse import bacc, mybir

    f32 = mybir.dt.float32
    f32r = mybir.dt.float32r
    EXP = mybir.ActivationFunctionType.Exp

    nc = bacc.Bacc("TRN2", target_bir_lowering=False, debug=False,
                   num_devices=N_CORES)
    xT = nc.dram_tensor("xT", [C, S], f32r, kind="ExternalInput").ap()
    wqT = nc.dram_tensor("wqT", [C, CPB], f32r, kind="ExternalInput").ap()
    wkT = nc.dram_tensor("wkT", [C, CPB], f32r, kind="ExternalInput").ap()
    wvT = nc.dram_tensor("wvT", [C, CPB], f32r, kind="ExternalInput").ap()
    wpT = nc.dram_tensor("wpT", [CPB, C], f32r, kind="ExternalInput").ap()
    y = nc.dram_tensor("y", [S, C], f32, kind="ExternalOutput").ap()

    KC = C // 128        # 8 contraction chunks for the projections
    NS = S // 512        # 4 free-dim chunks for q/k projections
    SC = S // 128        # 16 sequence chunks

    with tile.TileContext(nc) as tc:
        with (
            tc.tile_pool(name="xt", bufs=KC) as xt_pool,
            tc.tile_pool(name="wts", bufs=1) as w_pool,
            tc.tile_pool(name="qk", bufs=1) as qk_pool,
            tc.tile_pool(name="va", bufs=SC) as va_pool,
            tc.tile_pool(name="ot", bufs=1) as ot_pool,
            tc.tile_pool(name="exp", bufs=3) as exp_pool,
            tc.tile_pool(name="small", bufs=1) as small_pool,
            tc.tile_pool(name="ysb", bufs=2) as ysb_pool,
            tc.tile_pool(name="pse", bufs=2, space="PSUM") as psum_e,
            tc.tile_pool(name="pso", bufs=4, space="PSUM") as psum_o,
        ):
            # ---- load inputs ----
            xts = []
            for k in range(KC):
                t = xt_pool.tile([128, S], f32r, tag="xt", name=f"xt{k}")
                nc.sync.dma_start(t[:], xT[k * 128:(k + 1) * 128, :])
                xts.append(t)
            wq_t, wk_t, wv_t = [], [], []
            for name, dram, lst in (("wq", wqT, wq_t), ("wk", wkT, wk_t),
                                    ("wv", wvT, wv_t)):
                for k in range(KC):
                    t = w_pool.tile([128, CPB], f32r, tag="w", bufs=3 * KC,
                                    name=f"{name}{k}")
                    nc.sync.dma_start(t[:], dram[k * 128:(k + 1) * 128, :])
                    lst.append(t)
            wp_t = []
            for k in range(2):
                t = w_pool.tile([128, C], f32r, tag="wp", bufs=2,
                                name=f"wp{k}")
                nc.sync.dma_start(t[:], wpT[k * 128:(k + 1) * 128, :])
                wp_t.append(t)

            # ---- constants ----
            onesc = small_pool.tile([128, HPC], f32, tag="onesc")
            nc.vector.memset(onesc[:], 1.0)
            # ones row for the K=1 denominator-broadcast matmul
            ones_st = small_pool.tile([1, 64], f32, tag="ones_st")
            nc.vector.memset(ones_st[:], 1.0)
            ones64 = small_pool.tile([1, 64], f32r, tag="ones")
            nc.vector.tensor_copy(ones64[:], ones_st[:])

            # ---- q/k projections, transposed: qT/kT [CPB, S] ----
            qk_tiles = {}
            for nm, wts in (("q", wq_t), ("k", wk_t)):
                for m in range(2):
                    dst = qk_pool.tile([128, S], f32r, tag=f"{nm}{m}",
                                       name=f"{nm}T{m}")
                    qk_tiles[(nm, m)] = dst
                    for n in range(NS):
                        ps = psum_e.tile([128, 512], f32, tag="pse",
                                         name="psproj")
                        for k in range(KC):
                            nc.tensor.matmul(
                                ps[:],
                                wts[k][:, m * 128:(m + 1) * 128],
                                xts[k][:, n * 512:(n + 1) * 512],
                                start=(k == 0), stop=(k == KC - 1))
                        nc.vector.tensor_copy(dst[:, n * 512:(n + 1) * 512],
                                              ps[:])

            # ---- v projection, natural layout + ones column per head ----
            # va[sc] is [128, 4*65]: per head 64 v columns then a 1.0 column.
            va = []
            for sc in range(SC):
                t = va_pool.tile([128, HPC * 65], f32r, tag="va",
                                 name=f"va{sc}")
                tones = t[:].rearrange("p (h x) -> p h x", x=65)[:, :, 64:65]
                nc.vector.tensor_copy(
                    tones, onesc[:].rearrange("p (h x) -> p h x", x=1))
                ps = psum_e.tile([128, CPB], f32, tag="pse", name="psv")
                for k in range(KC):
                    nc.tensor.matmul(
                        ps[:],
                        xts[k][:, sc * 128:(sc + 1) * 128],
                        wv_t[k][:],
                        start=(k == 0), stop=(k == KC - 1))
                tv = t[:].rearrange("p (h x) -> p h x", x=65)[:, :, 0:64]
                pv = ps[:].rearrange("p (h d) -> p h d", d=64)
                nc.vector.tensor_copy(tv, pv)
                va.append(t)

            # ---- attention ----
            # Per (m, qv): accumulate PV + denominator in PSUM over allowed
            # kv views; exp batches both heads and both k-chunks of a kv view
            # in a single [128,1024] ACT op. Normalization is deferred per
            # qv-pair so the reciprocal runs 2-partition-wide, off the PE
            # critical path.
            ot_tiles = []
            for m in range(2):
                ot = ot_pool.tile([128, S], f32r, tag=f"ot{m}", name=f"oT{m}")
                ot_tiles.append(ot)
                kT_m = qk_tiles[("k", m)]
                qT_m = qk_tiles[("q", m)]
                if True:
                    for qv in range(V):
                        kvs = _allowed(qv)
                        qs = slice(qv * 256, (qv + 1) * 256)
                        pso = [psum_o.tile([65, 256], f32, tag="pso",
                                           name=f"pso{h}") for h in range(2)]
                        for i, kv in enumerate(kvs):
                            if SPLIT_ACT:
                                pssl = [psum_e.tile([128, 512], f32,
                                                    tag="pse", name="pss")
                                        for _ in range(2)]
                                pq = lambda h, j: pssl[h][:, j * 256:
                                                          (j + 1) * 256]
                            else:
                                pss = psum_e.tile([128, 1024], f32,
                                                  tag="pse", name="pss")
                                pq = lambda h, j: pss[:, (2 * h + j) * 256:
                                                      (2 * h + j + 1) * 256]
                            for h in range(2):
                                for j in range(2):
                                    kc = 2 * kv + j
                                    kw = ({"tile_position": (64 * h, 0)}
                                          if PACK_QK else {})
                                    nc.tensor.matmul(
                                        pq(h, j),
                                        kT_m[64 * h:64 * (h + 1),
                                             kc * 128:(kc + 1) * 128],
                                        qT_m[64 * h:64 * (h + 1), qs],
                                        start=True, stop=True, **kw)
                            et = exp_pool.tile([128, 1024], f32r, tag="exp")
                            if SPLIT_ACT:
                                for h in range(2):
                                    nc.scalar.activation(
                                        et[:, h * 512:(h + 1) * 512],
                                        pssl[h][:], EXP, scale=float(SCALE))
                            else:
                                nc.scalar.activation(et[:], pss[:], EXP,
                                                     scale=float(SCALE))
                            for h in range(2):
                                hh = 2 * m + h
                                for j in range(2):
                                    kc = 2 * kv + j
                                    nc.tensor.matmul(
                                        pso[h][:],
                                        va[kc][:, hh * 65:(hh + 1) * 65],
                                        et[:, (2 * h + j) * 256:
                                           (2 * h + j + 1) * 256],
                                        start=(i == 0 and j == 0),
                                        stop=(i == len(kvs) - 1 and j == 1))
                        # normalize: 1/denom, broadcast to 64 partitions
                        # via a K=1 PE matmul, multiply out of PSUM
                        for h in range(2):
                            rec = small_pool.tile([1, 256], f32, tag="rec",
                                                  bufs=4, name="rec")
                            if SAFE_RECIP:
                                nc.vector.reciprocal(rec[:],
                                                     pso[h][64:65, :])
                            else:
                                nc.vector.reciprocal_approx_fast(
                                    rec[:], pso[h][64:65, :])
                            recr = small_pool.tile([1, 256], f32r, tag="recr",
                                                   bufs=4, name="recr")
                            nc.vector.tensor_copy(recr[:], rec[:])
                            rb = psum_o.tile([64, 256], f32, tag="pso",
                                             name="rb")
                            nc.tensor.matmul(rb[:], ones64[:], recr[:],
                                             start=True, stop=True)
                            rbs = small_pool.tile([64, 256], f32, tag="rbs",
                                                  bufs=4, name="rbs")
                            nc.vector.tensor_copy(rbs[:], rb[:])
                            nc.vector.tensor_mul(ot[64 * h:64 * (h + 1), qs],
                                                 pso[h][0:64, :], rbs[:])

            # ---- output projection: y_partial = outT.T @ wpT ----
            for sc in range(SC):
                ys = ysb_pool.tile([128, C], f32, tag="ysb")
                for n in range(2):
                    ps = psum_e.tile([128, 512], f32, tag="pse", name="psy")
                    for k in range(2):
                        nc.tensor.matmul(
                            ps[:],
                            ot_tiles[k][:, sc * 128:(sc + 1) * 128],
                            wp_t[k][:, n * 512:(n + 1) * 512],
                            start=(k == 0), stop=(k == 1))
                    nc.vector.tensor_copy(ys[:, n * 512:(n + 1) * 512],
                                          ps[:])
                nc.sync.dma_start(y[sc * 128:(sc + 1) * 128, :], ys[:])

    nc.compile()
    return nc


def _get_compiled():
    if "nc" not in _compiled:
        _compiled["nc"] = build()
    return _compiled["nc"]


def make_in_maps(x, Wq, Wk, Wv, Wp):
    xf = np.asarray(x, np.float32).reshape(B, S, C)
    in_maps = []
    for c in range(N_CORES):
        b, g = divmod(c, HPC)
        hs = slice(g * CPB, (g + 1) * CPB)
        in_maps.append({
            "xT": np.ascontiguousarray(xf[b].T),
            "wqT": np.ascontiguousarray(np.asarray(Wq, np.float32)[hs].T),
            "wkT": np.ascontiguousarray(np.asarray(Wk, np.float32)[hs].T),
            "wvT": np.ascontiguousarray(np.asarray(Wv, np.float32)[hs].T),
            "wpT": np.ascontiguousarray(np.asarray(Wp, np.float32)[:, hs].T),
        })
    return in_maps


def kernel(x, Wq, Wk, Wv, Wp, bp, _trace=False, _tmpdir=None):
    global LAST_RESULTS
    from concourse import bass_utils

    nc = _get_compiled()
    in_maps = make_in_maps(x, Wq, Wk, Wv, Wp)
    kwargs = {}
    if _trace:
        kwargs = {"trace": True, "tmpdir": _tmpdir}
    res = bass_utils.run_bass_kernel_spmd(
        nc, in_maps, core_ids=list(range(N_CORES)), **kwargs)
    LAST_RESULTS = res
    yout = np.zeros((B, S, C), np.float32)
    for c in range(N_CORES):
        yout[c // HPC] += res.results[c]["y"]
    yout += np.asarray(bp, np.float32).reshape(1, 1, C)
    return yout.reshape(B, V, L, C)


# revision 13
# speedup vs baseline: 1.1366x; 1.1008x over previous
"""Block-sparse (view-causal) multi-head attention on 8 TRN2 NeuronCores.

Full inputs in, full output out. Sharding: data-parallel over batch (B=2),
tensor-parallel over heads (16 heads -> 4 per core). Each core computes its
4 heads' attention + its slice of the output projection; the host sums the
4 head-group partial projections per batch (the tensor-parallel reduce).

Device-side layout: activations kept transposed (qT/kT [dh, tokens]) so the
score matmuls need no transposes; V is augmented with a ones column so each
PV matmul accumulates both sum(exp*v) and the softmax denominator in PSUM.
All matmul operands are bf16 (PSUM accumulation in fp32): 1 cycle/row on
the PE plus fast weight load; measured end-to-end error vs the fp32
reference is ~4e-3 relative absmax.
"""

import sys

if "/opt/trn_rl_repo" not in sys.path:
    sys.path.insert(0, "/opt/trn_rl_repo")

import numpy as np
import ml_dtypes

B, V, L, C, H = 2, 8, 256, 1024, 16
S = V * L                # 2048 tokens
DH = C // H              # 64
HPC = 4                  # heads per core
CPB = HPC * DH           # 256 channel block per core
N_CORES = 8
SCALE = DH ** -0.5       # 1/8, folded into the exp activation

_compiled = {}
LAST_RESULTS = None
PACK_QK = True           # row-group pack the two heads' K=64 QK matmuls
SAFE_RECIP = False       # exact nc.vector.reciprocal instead of custom DVE
SPLIT_ACT = False        # two [128,512] exp ACTs instead of one [128,1024]


def _allowed(qv):
    """View-level mask row: views 0/1 cross-attend only; views >=2 block-causal."""
    if qv == 0:
        return [1]
    if qv == 1:
        return [0]
    return list(range(qv + 1))


def build():
    import concourse.tile as tile
    from concourse import bacc, mybir

    f32 = mybir.dt.float32
    bf16 = mybir.dt.bfloat16
    EXP = mybir.ActivationFunctionType.Exp

    nc = bacc.Bacc("TRN2", target_bir_lowering=False, debug=False,
                   num_devices=N_CORES)
    xT = nc.dram_tensor("xT", [C, S], bf16, kind="ExternalInput").ap()
    wqT = nc.dram_tensor("wqT", [C, CPB], bf16, kind="ExternalInput").ap()
    wkT = nc.dram_tensor("wkT", [C, CPB], bf16, kind="ExternalInput").ap()
    wvT = nc.dram_tensor("wvT", [C, CPB], bf16, kind="ExternalInput").ap()
    wpT = nc.dram_tensor("wpT", [CPB, C], bf16, kind="ExternalInput").ap()
    y = nc.dram_tensor("y", [S, C], f32, kind="ExternalOutput").ap()

    KC = C // 128        # 8 contraction chunks for the projections
    NS = S // 512        # 4 free-dim chunks for q/k projections
    SC = S // 128        # 16 sequence chunks

    with tile.TileContext(nc) as tc:
        with (
            tc.tile_pool(name="xt", bufs=KC) as xt_pool,
            tc.tile_pool(name="wts", bufs=1) as w_pool,
            tc.tile_pool(name="qk", bufs=1) as qk_pool,
            tc.tile_pool(name="va", bufs=SC) as va_pool,
            tc.tile_pool(name="ot", bufs=1) as ot_pool,
            tc.tile_pool(name="exp", bufs=3) as exp_pool,
            tc.tile_pool(name="small", bufs=1) as small_pool,
            tc.tile_pool(name="ysb", bufs=2) as ysb_pool,
            tc.tile_pool(name="pse", bufs=2, space="PSUM") as psum_e,
            tc.tile_pool(name="pso", bufs=4, space="PSUM") as psum_o,
        ):
            # ---- load inputs ----
            xts = []
            for k in range(KC):
                t = xt_pool.tile([128, S], bf16, tag="xt", name=f"xt{k}")
                nc.sync.dma_start(t[:], xT[k * 128:(k + 1) * 128, :])
                xts.append(t)
            wq_t, wk_t, wv_t = [], [], []
            for name, dram, lst in (("wq", wqT, wq_t), ("wk", wkT, wk_t),
                                    ("wv", wvT, wv_t)):
                for k in range(KC):
                    t = w_pool.tile([128, CPB], bf16, tag="w", bufs=3 * KC,
                                    name=f"{name}{k}")
                    nc.sync.dma_start(t[:], dram[k * 128:(k + 1) * 128, :])
                    lst.append(t)
            wp_t = []
            for k in range(2):
                t = w_pool.tile([128, C], bf16, tag="wp", bufs=2,
                                name=f"wp{k}")
                nc.sync.dma_start(t[:], wpT[k * 128:(k + 1) * 128, :])
                wp_t.append(t)

            # ---- constants ----
            onesc = small_pool.tile([128, HPC], bf16, tag="onesc")
            nc.vector.memset(onesc[:], 1.0)
            # ones row for the K=1 denominator-broadcast matmul
            ones64 = small_pool.tile([1, 64], bf16, tag="ones")
            nc.vector.memset(ones64[:], 1.0)

            # ---- q/k projections, transposed: qT/kT [CPB, S] ----
            qk_tiles = {}
            for nm, wts in (("q", wq_t), ("k", wk_t)):
                for m in range(2):
                    dst = qk_pool.tile([128, S], bf16, tag=f"{nm}{m}",
                                       name=f"{nm}T{m}")
                    qk_tiles[(nm, m)] = dst
                    for n in range(NS):
                        ps = psum_e.tile([128, 512], f32, tag="pse",
                                         name="psproj")
                        for k in range(KC):
                            nc.tensor.matmul(
                                ps[:],
                                wts[k][:, m * 128:(m + 1) * 128],
                                xts[k][:, n * 512:(n + 1) * 512],
                                start=(k == 0), stop=(k == KC - 1))
                        nc.vector.tensor_copy(dst[:, n * 512:(n + 1) * 512],
                                              ps[:])

            # ---- v projection, natural layout + ones column per head ----
            # va[sc] is [128, 4*65]: per head 64 v columns then a 1.0 column.
            va = []
            for sc in range(SC):
                t = va_pool.tile([128, HPC * 65], bf16, tag="va",
                                 name=f"va{sc}")
                tones = t[:].rearrange("p (h x) -> p h x", x=65)[:, :, 64:65]
                nc.vector.tensor_copy(
                    tones, onesc[:].rearrange("p (h x) -> p h x", x=1))
                ps = psum_e.tile([128, CPB], f32, tag="pse", name="psv")
                for k in range(KC):
                    nc.tensor.matmul(
                        ps[:],
                        xts[k][:, sc * 128:(sc + 1) * 128],
                        wv_t[k][:],
                        start=(k == 0), stop=(k == KC - 1))
                tv = t[:].rearrange("p (h x) -> p h x", x=65)[:, :, 0:64]
                pv = ps[:].rearrange("p (h d) -> p h d", d=64)
                nc.vector.tensor_copy(tv, pv)
                va.append(t)

            # ---- attention ----
            # Per (m, qv): accumulate PV + denominator in PSUM over allowed
            # kv views; exp batches both heads and both k-chunks of a kv view
            # in a single [128,1024] ACT op. Normalization is deferred per
            # qv-pair so the reciprocal runs 2-partition-wide, off the PE
            # critical path.
            ot_tiles = []
            for m in range(2):
                ot = ot_pool.tile([128, S], bf16, tag=f"ot{m}", name=f"oT{m}")
                ot_tiles.append(ot)
                kT_m = qk_tiles[("k", m)]
                qT_m = qk_tiles[("q", m)]
                if True:
                    for qv in range(V):
                        kvs = _allowed(qv)
                        qs = slice(qv * 256, (qv + 1) * 256)
                        pso = [psum_o.tile([65, 256], f32, tag="pso",
                                           name=f"pso{h}") for h in range(2)]
                        for i, kv in enumerate(kvs):
                            if SPLIT_ACT:
                                pssl = [psum_e.tile([128, 512], f32,
                                                    tag="pse", name="pss")
                                        for _ in range(2)]
                                pq = lambda h, j: pssl[h][:, j * 256:
                                                          (j + 1) * 256]
                            else:
                                pss = psum_e.tile([128, 1024], f32,
                                                  tag="pse", name="pss")
                                pq = lambda h, j: pss[:, (2 * h + j) * 256:
                                                      (2 * h + j + 1) * 256]
                            for h in range(2):
                                for j in range(2):
                                    kc = 2 * kv + j
                                    kw = ({"tile_position": (64 * h, 0)}
                                          if PACK_QK else {})
                                    nc.tensor.matmul(
                                        pq(h, j),
                                        kT_m[64 * h:64 * (h + 1),
                                             kc * 128:(kc + 1) * 128],
                                        qT_m[64 * h:64 * (h + 1), qs],
                                        start=True, stop=True, **kw)
                            et = exp_pool.tile([128, 1024], bf16, tag="exp")
                            if SPLIT_ACT:
                                for h in range(2):
                                    nc.scalar.activation(
                                        et[:, h * 512:(h + 1) * 512],
                                        pssl[h][:], EXP, scale=float(SCALE))
                            else:
                                nc.scalar.activation(et[:], pss[:], EXP,
                                                     scale=float(SCALE))
                            for h in range(2):
                                hh = 2 * m + h
                                for j in range(2):
                                    kc = 2 * kv + j
                                    nc.tensor.matmul(
                                        pso[h][:],
                                        va[kc][:, hh * 65:(hh + 1) * 65],
                                        et[:, (2 * h + j) * 256:
                                           (2 * h + j + 1) * 256],
                                        start=(i == 0 and j == 0),
                                        stop=(i == len(kvs) - 1 and j == 1))
                        # normalize: 1/denom, broadcast to 64 partitions
                        # via a K=1 PE matmul, multiply out of PSUM
                        for h in range(2):
                            rec = small_pool.tile([1, 256], f32, tag="rec",
                                                  bufs=4, name="rec")
                            nc.vector.reciprocal(rec[:], pso[h][64:65, :])
                            recr = small_pool.tile([1, 256], bf16, tag="recr",
                                                   bufs=4, name="recr")
                            nc.vector.tensor_copy(recr[:], rec[:])
                            rb = psum_o.tile([64, 256], f32, tag="pso",
                                             name="rb")
                            nc.tensor.matmul(rb[:], ones64[:], recr[:],
                                             start=True, stop=True)
                            rbs = small_pool.tile([64, 256], f32, tag="rbs",
                                                  bufs=4, name="rbs")
                            nc.vector.tensor_copy(rbs[:], rb[:])
                            nc.vector.tensor_mul(ot[64 * h:64 * (h + 1), qs],
                                                 pso[h][0:64, :], rbs[:])

            # ---- output projection: y_partial = outT.T @ wpT ----
            for sc in range(SC):
                ys = ysb_pool.tile([128, C], f32, tag="ysb")
                for n in range(2):
                    ps = psum_e.tile([128, 512], f32, tag="pse", name="psy")
                    for k in range(2):
                        nc.tensor.matmul(
                            ps[:],
                            ot_tiles[k][:, sc * 128:(sc + 1) * 128],
                            wp_t[k][:, n * 512:(n + 1) * 512],
                            start=(k == 0), stop=(k == 1))
                    nc.vector.tensor_copy(ys[:, n * 512:(n + 1) * 512],
                                          ps[:])
                nc.sync.dma_start(y[sc * 128:(sc + 1) * 128, :], ys[:])

    nc.compile()
    return nc


def _get_compiled():
    if "nc" not in _compiled:
        _compiled["nc"] = build()
    return _compiled["nc"]


def make_in_maps(x, Wq, Wk, Wv, Wp):
    xf = np.asarray(x, np.float32).reshape(B, S, C)
    in_maps = []
    for c in range(N_CORES):
        b, g = divmod(c, HPC)
        hs = slice(g * CPB, (g + 1) * CPB)
        bf = ml_dtypes.bfloat16
        in_maps.append({
            "xT": np.ascontiguousarray(xf[b].T).astype(bf),
            "wqT": np.ascontiguousarray(np.asarray(Wq, np.float32)[hs].T).astype(bf),
            "wkT": np.ascontiguousarray(np.asarray(Wk, np.float32)[hs].T).astype(bf),
            "wvT": np.ascontiguousarray(np.asarray(Wv, np.float32)[hs].T).astype(bf),
            "wpT": np.ascontiguousarray(np.asarray(Wp, np.float32)[:, hs].T).astype(bf),
        })
    return in_maps


def kernel(x, Wq, Wk, Wv, Wp, bp, _trace=False, _tmpdir=None):
    global LAST_RESULTS
    from concourse import bass_utils

    nc = _get_compiled()
    in_maps = make_in_maps(x, Wq, Wk, Wv, Wp)
    kwargs = {}
    if _trace:
        kwargs = {"trace": True, "tmpdir": _tmpdir}
    res = bass_utils.run_bass_kernel_spmd(
        nc, in_maps, core_ids=list(range(N_CORES)), **kwargs)
    LAST_RESULTS = res
    yout = np.zeros((B, S, C), np.float32)
    for c in range(N_CORES):
        yout[c // HPC] += res.results[c]["y"]
    yout += np.asarray(bp, np.float32).reshape(1, 1, C)
    return yout.reshape(B, V, L, C)


# revision 17
# speedup vs baseline: 1.6176x; 1.4232x over previous
"""Block-sparse (view-causal) multi-head attention on 8 TRN2 NeuronCores.

Full inputs in, full output out. Sharding: data-parallel over batch (B=2),
tensor-parallel over heads (16 heads -> 4 per core). Each core computes its
4 heads' attention + its slice of the output projection; the host sums the
4 head-group partial projections per batch (the tensor-parallel reduce).

Device-side layout: activations kept transposed (qT/kT [dh, tokens]) so the
score matmuls need no transposes; V is augmented with a ones column so each
PV matmul accumulates both sum(exp*v) and the softmax denominator in PSUM.
All matmul operands are bf16 (PSUM accumulation in fp32): 1 cycle/row on
the PE plus fast weight load; measured end-to-end error vs the fp32
reference is ~4e-3 relative absmax.
"""

import sys

if "/opt/trn_rl_repo" not in sys.path:
    sys.path.insert(0, "/opt/trn_rl_repo")

import numpy as np
import ml_dtypes

B, V, L, C, H = 2, 8, 256, 1024, 16
S = V * L                # 2048 tokens
DH = C // H              # 64
HPC = 4                  # heads per core
CPB = HPC * DH           # 256 channel block per core
N_CORES = 8
SCALE = DH ** -0.5       # 1/8, folded into the exp activation

_compiled = {}
LAST_RESULTS = None
PACK_QK = True           # row-group pack the two heads' K=64 QK matmuls
SAFE_RECIP = False       # exact nc.vector.reciprocal instead of custom DVE
SPLIT_ACT = False        # two [128,512] exp ACTs instead of one [128,1024]


def _allowed(qv):
    """View-level mask row: views 0/1 cross-attend only; views >=2 block-causal."""
    if qv == 0:
        return [1]
    if qv == 1:
        return [0]
    return list(range(qv + 1))


def build():
    import concourse.tile as tile
    from concourse import bacc, mybir

    f32 = mybir.dt.float32
    bf16 = mybir.dt.bfloat16
    EXP = mybir.ActivationFunctionType.Exp

    nc = bacc.Bacc("TRN2", target_bir_lowering=False, debug=False,
                   num_devices=N_CORES)
    xT = nc.dram_tensor("xT", [C, S], bf16, kind="ExternalInput").ap()
    wqT = nc.dram_tensor("wqT", [C, CPB], bf16, kind="ExternalInput").ap()
    wkT = nc.dram_tensor("wkT", [C, CPB], bf16, kind="ExternalInput").ap()
    wvT = nc.dram_tensor("wvT", [C, CPB], bf16, kind="ExternalInput").ap()
    wpT = nc.dram_tensor("wpT", [CPB, C], bf16, kind="ExternalInput").ap()
    y = nc.dram_tensor("y", [S, C], f32, kind="ExternalOutput").ap()

    KC = C // 128        # 8 contraction chunks for the projections
    NS = S // 512        # 4 free-dim chunks for q/k projections
    SC = S // 128        # 16 sequence chunks

    with tile.TileContext(nc) as tc:
        with (
            tc.tile_pool(name="xt", bufs=KC) as xt_pool,
            tc.tile_pool(name="wts", bufs=1) as w_pool,
            tc.tile_pool(name="qk", bufs=1) as qk_pool,
            tc.tile_pool(name="va", bufs=SC) as va_pool,
            tc.tile_pool(name="ot", bufs=1) as ot_pool,
            tc.tile_pool(name="exp", bufs=3) as exp_pool,
            tc.tile_pool(name="small", bufs=1) as small_pool,
            tc.tile_pool(name="ysb", bufs=2) as ysb_pool,
            tc.tile_pool(name="pse", bufs=2, space="PSUM") as psum_e,
            tc.tile_pool(name="pso", bufs=4, space="PSUM") as psum_o,
        ):
            # ---- load inputs ----
            xts = []
            for k in range(KC):
                t = xt_pool.tile([128, S], bf16, tag="xt", name=f"xt{k}")
                nc.sync.dma_start(t[:], xT[k * 128:(k + 1) * 128, :])
                xts.append(t)
            wq_t, wk_t, wv_t = [], [], []
            for name, dram, lst in (("wq", wqT, wq_t), ("wk", wkT, wk_t),
                                    ("wv", wvT, wv_t)):
                for k in range(KC):
                    t = w_pool.tile([128, CPB], bf16, tag="w", bufs=3 * KC,
                                    name=f"{name}{k}")
                    nc.sync.dma_start(t[:], dram[k * 128:(k + 1) * 128, :])
                    lst.append(t)
            wp_t = []
            for k in range(2):
                t = w_pool.tile([128, C], bf16, tag="wp", bufs=2,
                                name=f"wp{k}")
                nc.sync.dma_start(t[:], wpT[k * 128:(k + 1) * 128, :])
                wp_t.append(t)

            # ---- constants ----
            onesc = small_pool.tile([128, HPC], bf16, tag="onesc")
            nc.vector.memset(onesc[:], 1.0)
            # identity for PE-mode transposes (attention out -> outT)
            ident = small_pool.tile([128, 128], bf16, tag="ident")
            from concourse.masks import make_identity
            make_identity(nc, ident[:])

            # ---- q/k projections, transposed: qT/kT [CPB, S] ----
            qk_tiles = {}
            for nm, wts in (("q", wq_t), ("k", wk_t)):
                for m in range(2):
                    dst = qk_pool.tile([128, S], bf16, tag=f"{nm}{m}",
                                       name=f"{nm}T{m}")
                    qk_tiles[(nm, m)] = dst
                    for n in range(NS):
                        ps = psum_e.tile([128, 512], f32, tag="pse",
                                         name="psproj")
                        for k in range(KC):
                            nc.tensor.matmul(
                                ps[:],
                                wts[k][:, m * 128:(m + 1) * 128],
                                xts[k][:, n * 512:(n + 1) * 512],
                                start=(k == 0), stop=(k == KC - 1))
                        nc.vector.tensor_copy(dst[:, n * 512:(n + 1) * 512],
                                              ps[:])

            # ---- v projection, natural layout + ones column per head ----
            # va[sc] is [128, 4*65]: per head 64 v columns then a 1.0 column.
            va = []
            for sc in range(SC):
                t = va_pool.tile([128, HPC * 65], bf16, tag="va",
                                 name=f"va{sc}")
                tones = t[:].rearrange("p (h x) -> p h x", x=65)[:, :, 64:65]
                nc.vector.tensor_copy(
                    tones, onesc[:].rearrange("p (h x) -> p h x", x=1))
                ps = psum_e.tile([128, CPB], f32, tag="pse", name="psv")
                for k in range(KC):
                    nc.tensor.matmul(
                        ps[:],
                        xts[k][:, sc * 128:(sc + 1) * 128],
                        wv_t[k][:],
                        start=(k == 0), stop=(k == KC - 1))
                tv = t[:].rearrange("p (h x) -> p h x", x=65)[:, :, 0:64]
                pv = ps[:].rearrange("p (h d) -> p h d", d=64)
                nc.vector.tensor_copy(tv, pv)
                va.append(t)

            # ---- attention ----
            # Flipped PV: out_psum [q, 4*(dh+1)] = expT.T @ va, so the
            # softmax denominator lands as a column -> partition-parallel
            # reciprocal + tensor_scalar normalize. The natural-layout
            # result is PE-transposed back to outT for the projection.
            on_tiles = []
            for sc in range(SC):
                t = small_pool.tile([128, CPB], bf16, tag="on",
                                    bufs=SC, name=f"on{sc}")
                on_tiles.append(t)
            ot_tiles = []
            for m in range(2):
                ot = ot_pool.tile([128, S], bf16, tag=f"ot{m}", name=f"oT{m}")
                ot_tiles.append(ot)
            for m in range(2):
                kT_m = qk_tiles[("k", m)]
                qT_m = qk_tiles[("q", m)]
                for qv in range(V):
                    kvs = _allowed(qv)
                    qs = slice(qv * 256, (qv + 1) * 256)
                    ets = []
                    for kv in kvs:
                        pss = psum_e.tile([128, 1024], f32, tag="pse",
                                          name="pss")
                        for h in range(2):
                            for j in range(2):
                                kc = 2 * kv + j
                                nc.tensor.matmul(
                                    pss[:, (2 * h + j) * 256:
                                        (2 * h + j + 1) * 256],
                                    kT_m[64 * h:64 * (h + 1),
                                         kc * 128:(kc + 1) * 128],
                                    qT_m[64 * h:64 * (h + 1), qs],
                                    start=True, stop=True)
                        et = exp_pool.tile([128, 1024], bf16, tag="exp",
                                           bufs=10)
                        nc.scalar.activation(et[:], pss[:], EXP,
                                             scale=float(SCALE))
                        ets.append((kv, et))
                    # PV: one accumulation group per (h, qc), each group
                    # owning its PSUM bank exclusively while open (start
                    # marks the whole 2KB zero-region)
                    rp = small_pool.tile([128, 4], f32, tag="rp", bufs=4,
                                         name="rp")
                    for h in range(2):
                        hh = 2 * m + h
                        for qc in range(2):
                            g = 2 * h + qc
                            pg = psum_o.tile([128, 65], f32, tag="pso",
                                             name=f"pg{g}")
                            for i, (kv, et) in enumerate(ets):
                                for j in range(2):
                                    kc = 2 * kv + j
                                    nc.tensor.matmul(
                                        pg[:],
                                        et[:, (2 * h + j) * 256 + qc * 128:
                                           (2 * h + j) * 256 + qc * 128
                                           + 128],
                                        va[kc][:, hh * 65:(hh + 1) * 65],
                                        start=(i == 0 and j == 0),
                                        stop=(i == len(kvs) - 1 and j == 1))
                            sc = qv * 2 + qc
                            nc.vector.reciprocal(rp[:, g:g + 1],
                                                 pg[:, 64:65])
                            nc.vector.tensor_scalar_mul(
                                on_tiles[sc][:, hh * 64:(hh + 1) * 64],
                                pg[:, 0:64],
                                rp[:, g:g + 1])
            # transpose natural-layout output back to outT [c', s]
            for sc in range(SC):
                for half in range(2):
                    pt = psum_o.tile([128, 128], bf16, tag="pso", name="pt")
                    nc.tensor.transpose(
                        pt[:], on_tiles[sc][:, half * 128:(half + 1) * 128],
                        ident[:])
                    nc.vector.tensor_copy(
                        ot_tiles[half][:, sc * 128:(sc + 1) * 128], pt[:])

            # ---- output projection: y_partial = outT.T @ wpT ----
            for sc in range(SC):
                ys = ysb_pool.tile([128, C], f32, tag="ysb")
                for n in range(2):
                    ps = psum_e.tile([128, 512], f32, tag="pse", name="psy")
                    for k in range(2):
                        nc.tensor.matmul(
                            ps[:],
                            ot_tiles[k][:, sc * 128:(sc + 1) * 128],
                            wp_t[k][:, n * 512:(n + 1) * 512],
                            start=(k == 0), stop=(k == 1))
                    nc.vector.tensor_copy(ys[:, n * 512:(n + 1) * 512],
                                          ps[:])
                nc.sync.dma_start(y[sc * 128:(sc + 1) * 128, :], ys[:])

    nc.compile()
    return nc


def _get_compiled():
    if "nc" not in _compiled:
        _compiled["nc"] = build()
    return _compiled["nc"]


def make_in_maps(x, Wq, Wk, Wv, Wp):
    xf = np.asarray(x, np.float32).reshape(B, S, C)
    in_maps = []
    for c in range(N_CORES):
        b, g = divmod(c, HPC)
        hs = slice(g * CPB, (g + 1) * CPB)
        bf = ml_dtypes.bfloat16
        in_maps.append({
            "xT": np.ascontiguousarray(xf[b].T).astype(bf),
            "wqT": np.ascontiguousarray(np.asarray(Wq, np.float32)[hs].T).astype(bf),
            "wkT": np.ascontiguousarray(np.asarray(Wk, np.float32)[hs].T).astype(bf),
            "wvT": np.ascontiguousarray(np.asarray(Wv, np.float32)[hs].T).astype(bf),
            "wpT": np.ascontiguousarray(np.asarray(Wp, np.float32)[:, hs].T).astype(bf),
        })
    return in_maps


def kernel(x, Wq, Wk, Wv, Wp, bp, _trace=False, _tmpdir=None):
    global LAST_RESULTS
    from concourse import bass_utils

    nc = _get_compiled()
    in_maps = make_in_maps(x, Wq, Wk, Wv, Wp)
    kwargs = {}
    if _trace:
        kwargs = {"trace": True, "tmpdir": _tmpdir}
    res = bass_utils.run_bass_kernel_spmd(
        nc, in_maps, core_ids=list(range(N_CORES)), **kwargs)
    LAST_RESULTS = res
    yout = np.zeros((B, S, C), np.float32)
    for c in range(N_CORES):
        yout[c // HPC] += res.results[c]["y"]
    yout += np.asarray(bp, np.float32).reshape(1, 1, C)
    return yout.reshape(B, V, L, C)
